# revision 1
# baseline (speedup 1.0000x reference)
"""L1-distance attention on 8 Trainium2 NeuronCores.

attn[b,s,t,h] = -sum_w |q[b,s,h,w] - k[b,t,h,w]| / sqrt(w),  B=1, S=T=1024, H=8, W=32.

Algorithm (per core, cores shard t into 8 blocks of 128):
  |a-b| = 2*max(a,b) - a - b, so
  sum_w |q-k| = 2*sum_w max(q_w, k_w) - Qs[s,h] - Kt[t,h]
with Qs = sum_w q, Kt = sum_w k.

Per core layout: partitions p = 32*ts + w (ts in [0,4), w in [0,32)); t_local = 32*ts + tb.
  stage 1 (DVE, bf16 4x): M[tb,h][p, s] = max(q[s,h,w(p)], k[t(p,tb),h,w(p)])
     via tensor_scalar(max) with q streamed [128,1024] and k as per-partition scalar.
  stage 2 (PE): PSUM[m, s] accumulates 32 selector matmuls (entries 2.0) mapping
     (ts, tb mod 8, h mod 4) -> m, plus one K=4 matmul adding -Qs[s, h].
  evac (ACT): out = Identity(psum * (-1/sqrt(32)) + scale*Kt[m])  -> SBUF -> DRAM.

Host: pure layout prep (transpose/cast/replicate) + final reassembly transpose.
"""
import numpy as np
import ml_dtypes

import concourse.bacc as bacc
import concourse.tile as tile
import concourse.mybir as mybir
from concourse.bass_utils import run_bass_kernel_spmd

BF16 = ml_dtypes.bfloat16
SCALE = float(1.0 / np.sqrt(32.0))
NCORES = 8
S = 1024   # queries (full, on free dim)
TC = 128   # keys per core
H = 8
W = 32

LAST_RESULTS = None  # test harness reads exec_time_ns from here

_nc_cache = None


def _build_program():
    A = mybir.AluOpType
    F = mybir.ActivationFunctionType
    bf = mybir.dt.bfloat16
    f32 = mybir.dt.float32

    nc = bacc.Bacc("TRN2", target_bir_lowering=False)

    qt_d = nc.dram_tensor("qt", [H, 128, S], bf, kind="ExternalInput")
    ks_d = nc.dram_tensor("ks", [H, 128, 32], f32, kind="ExternalInput")
    sel_d = nc.dram_tensor("sel", [32, 128, 128], bf, kind="ExternalInput")
    selq_d = nc.dram_tensor("selq", [4, 128], bf, kind="ExternalInput")
    qsw_d = nc.dram_tensor("qsw", [128, 64, W], bf, kind="ExternalInput")
    ktw_d = nc.dram_tensor("ktw", [8, 128, W], f32, kind="ExternalInput")
    out_d = nc.dram_tensor("out", [16, 128, 512], f32, kind="ExternalOutput")
    qs_stage = nc.dram_tensor("qs_stage", [H, S], bf)  # internal bounce

    with tile.TileContext(nc) as tc:
        with tc.tile_pool(name="singles", bufs=1) as sg, \
             tc.tile_pool(name="mpool", bufs=2) as mp, \
             tc.tile_pool(name="evp", bufs=4) as evp, \
             tc.tile_pool(name="psp", bufs=4, space="PSUM") as psp:

            qt_s = []
            ks_s = []
            for h in range(H):
                t = sg.tile([128, S], bf, tag=f"qt{h}")
                nc.sync.dma_start(out=t, in_=qt_d[h])
                qt_s.append(t)
                t2 = sg.tile([128, 32], f32, tag=f"ks{h}")
                nc.sync.dma_start(out=t2, in_=ks_d[h])
                ks_s.append(t2)
            sel_s = []
            for j in range(32):
                t = sg.tile([128, 128], bf, tag=f"sel{j}")
                nc.sync.dma_start(out=t, in_=sel_d[j])
                sel_s.append(t)
            selq_s = sg.tile([4, 128], bf, tag="selq")
            nc.sync.dma_start(out=selq_s, in_=selq_d[:])

            # ---- Qs = sum_w q on device: reduce, negate-cast, bounce, reload
            qsw_s = sg.tile([128, 64, W], bf, tag="qsw")
            nc.sync.dma_start(out=qsw_s, in_=qsw_d[:])
            qs_red = sg.tile([128, 64], f32, tag="qsred")
            nc.vector.tensor_reduce(qs_red[:], qsw_s[:], axis=mybir.AxisListType.X,
                                    op=A.add)
            qs_neg = sg.tile([128, 64], bf, tag="qsneg")
            nc.vector.tensor_scalar(out=qs_neg[:], in0=qs_red[:], scalar1=-1.0,
                                    scalar2=None, op0=A.mult)
            qs_view = qs_stage[:].rearrange("h (sb sp) -> (h sb) sp", sp=64)
            nc.sync.dma_start(out=qs_view, in_=qs_neg[:])
            qs_sb = []
            for hB in range(2):
                t = sg.tile([4, S], bf, tag=f"qs{hB}")
                nc.sync.dma_start(out=t, in_=qs_stage[4 * hB:4 * hB + 4, :])
                qs_sb.append(t)

            # ---- Kt bias per (tbB, hB): scale * sum_w k
            kt_bias = []
            for g2 in range(8):
                ktw_s = sg.tile([128, W], f32, tag=f"ktw{g2}")
                nc.sync.dma_start(out=ktw_s, in_=ktw_d[g2])
                red = sg.tile([128, 1], f32, tag=f"ktr{g2}")
                nc.vector.tensor_reduce(red[:], ktw_s[:], axis=mybir.AxisListType.X,
                                        op=A.add)
                bias = sg.tile([128, 1], f32, tag=f"ktb{g2}")
                nc.vector.tensor_scalar(out=bias[:], in0=red[:], scalar1=SCALE,
                                        scalar2=None, op0=A.mult)
                kt_bias.append(bias)

            # ---- main pipeline
            for tbB in range(4):
                for hB in range(2):
                    g2 = tbB * 2 + hB
                    m_tiles = {}
                    for b in range(4):
                        h = 4 * hB + b
                        for a in range(8):
                            tb = 8 * tbB + a
                            mt = mp.tile([128, S], bf, tag=f"M{a}_{b}")
                            nc.vector.tensor_scalar(
                                out=mt[:], in0=qt_s[h][:],
                                scalar1=ks_s[h][:, tb:tb + 1], scalar2=None,
                                op0=A.max)
                            m_tiles[(a, b)] = mt
                    for sc in range(2):
                        g = g2 * 2 + sc
                        psum = psp.tile([128, 512], f32, tag="ps")
                        for j in range(32):
                            a, b = j % 8, j // 8
                            nc.tensor.matmul(
                                psum[:], sel_s[j][:],
                                m_tiles[(a, b)][:, 512 * sc:512 * (sc + 1)],
                                start=(j == 0), stop=False)
                        nc.tensor.matmul(
                            psum[:], selq_s[:],
                            qs_sb[hB][:, 512 * sc:512 * (sc + 1)],
                            start=False, stop=True)
                        ev = evp.tile([128, 512], f32, tag="ev")
                        nc.scalar.activation(ev[:], psum[:], F.Identity,
                                             bias=kt_bias[g2][:], scale=-SCALE)
                        nc.sync.dma_start(out=out_d[g], in_=ev[:])

    nc.compile()
    return nc


def _prep_inputs(q, k):
    """Pure layout prep. q, k: [1, 1024, 8, 32] fp32 (numpy)."""
    q = np.asarray(q)[0]  # [S, H, W]
    k = np.asarray(k)[0]  # [T, H, W]

    # qt[h, 32*ts+w, s] = q[s, h, w], ts-replicated
    qt = np.ascontiguousarray(
        np.tile(q.transpose(1, 2, 0), (1, 4, 1))).astype(BF16)  # [H, 128, S]

    # qsw[(h, sb), s', w] = q[64*sb + s', h, w]
    qsw = np.ascontiguousarray(
        q.reshape(16, 64, H, W).transpose(2, 0, 1, 3).reshape(128, 64, W)
    ).astype(BF16)

    # selectors
    sel = np.zeros((32, 128, 128), dtype=BF16)
    for j in range(32):
        a, b = j % 8, j // 8
        m = 4 * a + 32 * b
        for ts in range(4):
            for w in range(W):
                sel[j, 32 * ts + w, m + ts] = 2.0
    selq = np.zeros((4, 128), dtype=BF16)
    for mm in range(128):
        selq[mm // 32, mm] = 1.0

    in_maps = []
    for c in range(NCORES):
        kc = k[128 * c:128 * (c + 1)]  # [128 t_local, H, W]
        # ks[h, 32*ts+w, tb] = kc[32*ts + tb, h, w]
        k4 = kc.reshape(4, 32, H, W)  # [ts, tb, h, w]
        ks = np.ascontiguousarray(k4.transpose(2, 0, 3, 1).reshape(H, 128, 32)
                                  ).astype(np.float32)
        # ktw[(tbB, hB)][m = ts+4a+32b, w] = kc[32*ts + 8*tbB + a, 4*hB + b, w]
        ktw = np.empty((8, 128, W), dtype=np.float32)
        for tbB in range(4):
            for hB in range(2):
                blk = k4[:, 8 * tbB:8 * tbB + 8, 4 * hB:4 * hB + 4, :]  # [ts,a,b,w]
                ktw[tbB * 2 + hB] = blk.transpose(2, 1, 0, 3).reshape(128, W)
        in_maps.append({"qt": qt, "ks": ks, "sel": sel, "selq": selq,
                        "qsw": qsw, "ktw": ktw})
    return in_maps


def kernel(q, k):
    global _nc_cache, LAST_RESULTS
    if _nc_cache is None:
        _nc_cache = _build_program()
    nc = _nc_cache

    in_maps = _prep_inputs(q, k)
    res = run_bass_kernel_spmd(nc, in_maps, core_ids=list(range(NCORES)))
    LAST_RESULTS = res

    out = np.empty((1, S, 1024, H), dtype=np.float32)
    for c in range(NCORES):
        r = res.results[c]["out"]  # [16, 128, 512]
        arr = r.reshape(4, 2, 2, 4, 8, 4, 512)  # [tbB, hB, sc, b, a, ts, s']
        # -> [ (sc, s'), (ts, tbB, a), (hB, b) ] = [s, t_local, h]
        blk = arr.transpose(2, 6, 5, 0, 4, 1, 3).reshape(S, 128, H)
        out[0, :, 128 * c:128 * (c + 1), :] = blk
    return out
